# revision 29
# baseline (speedup 1.0000x reference)
"""Trainium2 Bass kernel for nn_Basis: Gram-Schmidt orthonormalization of a
500000x64 matrix across 8 NeuronCores.

Math: classical Gram-Schmidt of a (very well conditioned) Gaussian matrix
equals the QR Q-factor with positive-diagonal R, which equals Phi @ W where
W = R^{-1} = inverse Cholesky factor of G = Phi^T Phi.  W is computed with a
quadratically-convergent coupled iteration on the Gram matrix (no 64-step
sequential scan):  A = G/m = I + E with ||E|| ~ 2%, iterate
W <- W (I - Omega(W^T A W - I)) where Omega(F) = strict_upper(F) + diag(F)/2.
One iteration reaches ~1e-6; the fp16 data path dominates the error budget.

v2 (fp16 I/O): Phi is cast to fp16 on the host, halving DMA-in; Q^T leaves
the chip as fp16 (unscaled by sqrt(m) so values are O(1)), halving DMA-out.
Rows are packed 4 per partition (GROUP=512) so fp16 DMA chunks stay 512B.
The Gram accumulates into a single 64x64 PSUM tile via 4 narrow matmuls per
group (no even/odd block merge).  The Newton iteration runs on 128x128
block-diagonal operands so the stationary matrix for phase 3 comes out in
block form directly.  PE transposes of Phi tiles are split between phase A
(load), the AllReduce window, and the phase-3 stage loop so every phase
stays under the DMA roofline.  The host de-interleaves and scales output.
"""
import sys

sys.path.insert(0, "/opt/trn_rl_repo")

import numpy as np

N_CORES = 8
M_FULL = 500000
KAP = 64
R_CORE = M_FULL // N_CORES          # 62500 rows per core
GROUP = 512                          # rows per packed group (4 per partition)
TILE_G = 4                           # groups per DMA tile
N_TILES = 31
N_GROUPS = N_TILES * TILE_G          # 124
R_PAD = N_GROUPS * GROUP             # 63488 padded rows per core
XCOLS = N_GROUPS * 2 * 128           # 31744 columns in phit / qt
STAGE = 2048                         # output columns per staged DMA
A_TR = 12                            # tiles transposed inline during load
MID_TR = 10                          # tiles transposed during the AllGather
N_ITERS = 0

_CACHE = {}


COLL_MODE = "agsh"


def _build_nc(n_tiles=N_TILES, m_scale=float(M_FULL), n_cores=N_CORES,
              collective=True, ablate=(), repeat=1,
              a_tr=A_TR, mid_tr=MID_TR, n_iters=N_ITERS, coll=None):
    if coll is None:
        coll = COLL_MODE
    if n_iters > 0 or not collective:
        coll = "ar"
    import concourse.mybir as mybir
    from concourse import bacc, tile

    r_pad = n_tiles * TILE_G * GROUP
    xcols = n_tiles * TILE_G * 2 * 128
    # The last group (rows 62976..63487) is pure zero padding: skip its load,
    # gram, transposes and output columns. Host decode never reads past row
    # 62975 (group 122), and the donated output buffer zero-fills the tail.
    skip_last = n_tiles == N_TILES and "p3" not in ablate
    xcols_out = xcols - (256 if skip_last else 0)

    f32 = mybir.dt.float32
    f16 = mybir.dt.float16
    nc = bacc.Bacc(trn_type="TRN2", target_bir_lowering=False, debug=False)

    phi = nc.declare_dram_parameter("phi", [r_pad, KAP], f16, isOutput=False)
    identh = nc.declare_dram_parameter("identh", [128, 128], f16, isOutput=False)
    # consts128 cols: [0:128]=I(f32), [128:256]=1.5I, [256:384]=blockdiag
    # Omega mask (strict upper 1.0, diag 0.5) scaled by 1/m, [384:512]=0.5I,
    # [512:640]=unscaled Omega mask (Newton iterations only),
    # [640:704]=vertically stacked [Omega/m; Omega/m] for the allgather path
    consts = nc.declare_dram_parameter("consts128", [128, 704], f32, isOutput=False)
    qt = nc.declare_dram_parameter("qt", [128, xcols], f16, isOutput=True)

    from concourse.bass import ds

    # phi rows: row = 512*(TILE_G*t+g) + 4p + j, col c -> per tile [p, g, (j c)]
    def phi_tile_ap(t, ng=TILE_G):
        return phi[ds(t * TILE_G * GROUP, ng * GROUP), :].rearrange(
            "(g p j) c -> p g (j c)", g=ng, p=128, j=4
        )

    with tile.TileContext(nc) as tc:
        with (
            tc.tile_pool(name="consts", bufs=1) as cpool,
            tc.tile_pool(name="persist", bufs=1) as persist,
            tc.tile_pool(name="inp", bufs=n_tiles) as inp,
            tc.tile_pool(name="outp", bufs=3) as outp,
            tc.tile_pool(name="small", bufs=2) as small,
            tc.tile_pool(name="ps_gram", bufs=1, space="PSUM") as ps_gram,
            tc.tile_pool(name="ps_tr", bufs=2, space="PSUM") as ps_tr,
            tc.tile_pool(name="ps_it", bufs=2, space="PSUM") as ps_it,
            tc.tile_pool(name="ps_qt", bufs=3, space="PSUM") as ps_qt,
            tc.tile_pool(name="dram", bufs=1, space="DRAM") as dram,
        ):
            ident_sb = cpool.tile([128, 128], f16)
            nc.sync.dma_start(ident_sb, identh[:, :])
            consts_sb = cpool.tile([128, 704], f32)
            nc.sync.dma_start(consts_sb, consts[:, :])
            eyef = consts_sb[:, 0:128]
            eye15 = consts_sb[:, 128:256]
            mhalf_m = consts_sb[:, 256:384]
            ehalf = consts_sb[:, 384:512]
            mhalf = consts_sb[:, 512:640]
            mh2m = consts_sb[:, 640:704]

            for _rep in range(repeat):
                phit = persist.tile([128, xcols], f16)
                a_blk = persist.tile([128, 128], f32, tag="ablk")
                nc.any.memzero(a_blk)
                g64 = ps_gram.tile([64, 64], f32)

                # ---- Phase A: load + gram; transpose first a_tr tiles ----
                n_groups = n_tiles * TILE_G
                gi = 0

                def emit_transposes(t, in_t, evac_parity, ng=TILE_G,
                                    evac_eng=None):
                    # 2 transposes per group -> 2 PSUM tiles -> 2 evacs.
                    # evac_eng None alternates DVE/ACT; "act" keeps DVE clear
                    # (used in the collective window so the reduce chain isn't
                    # queued behind evacuations on DVE).
                    for half_t in range(2):
                        nk = min(4, 2 * ng - 4 * half_t)
                        if nk <= 0:
                            break
                        tr_ps = ps_tr.tile([128, 512], f16)
                        for k in range(nk):
                            g = half_t * 2 + k // 2
                            h = k % 2
                            nc.tensor.transpose(
                                tr_ps[:, 128 * k : 128 * (k + 1)],
                                in_t[:, g, 128 * h : 128 * (h + 1)],
                                ident_sb,
                            )
                        x0 = 1024 * t + 512 * half_t
                        dst = phit[:, x0 : x0 + 128 * nk]
                        use_dve = (
                            evac_eng is None and (evac_parity + half_t) % 2 == 0
                        )
                        if use_dve:
                            nc.vector.tensor_copy(dst, tr_ps[:, : 128 * nk])
                        else:
                            nc.scalar.copy(dst, tr_ps[:, : 128 * nk])

                tiles = []
                tile_ng = [
                    TILE_G - 1 if (skip_last and t == n_tiles - 1) else TILE_G
                    for t in range(n_tiles)
                ]
                n_gram = 4 * sum(tile_ng)
                for t in range(n_tiles):
                    ng = tile_ng[t]
                    in_t = inp.tile([128, TILE_G, 256], f16)
                    nc.sync.dma_start(in_t[:, :ng, :], phi_tile_ap(t, ng))
                    tiles.append(in_t)
                    for g in range(ng):
                        for j in range(4):
                            if "gram" in ablate:
                                gi += 1
                                continue
                            nc.tensor.matmul(
                                g64,
                                in_t[:, g, 64 * j : 64 * j + 64],
                                in_t[:, g, 64 * j : 64 * j + 64],
                                start=(gi == 0),
                                stop=(gi == n_gram - 1),
                            )
                            gi += 1
                    if "tr" not in ablate and t < a_tr:
                        emit_transposes(t, tiles[t], t, ng)

                # ---- combine the per-core 64x64 Grams across cores ----
                g_in = dram.tile([64, 64], f32)
                g_out = dram.tile([64, 64], f32)
                wblk = persist.tile([128, 128], f16, tag="wblk")
                if coll in ("ag", "agsh"):
                    nc.any.memzero(wblk)
                if "gram" not in ablate:
                    g_sb = small.tile([64, 64], f32, tag="gsb")
                    nc.vector.tensor_copy(g_sb, g64)
                    nc.sync.dma_start(g_in[:], g_sb)
                if coll in ("ag", "agsh"):
                    # AllGather + on-chip reduce: one collective phase instead
                    # of reduce-scatter+gather, and no DRAM round trip for the
                    # block-diagonal A.
                    ag_out = dram.tile(
                        [64 * n_cores, 64], f32, tag="agout",
                        addr_space="Shared" if coll == "agsh" else "Local",
                    )
                    nc.gpsimd.collective_compute(
                        "AllGather",
                        mybir.AluOpType.bypass,
                        replica_groups=[list(range(n_cores))],
                        ins=[g_in.opt()],
                        outs=[ag_out.opt()],
                    )
                    mid_end = min(a_tr + mid_tr, n_tiles)
                    if "tr" not in ablate:
                        for t in range(a_tr, mid_end):
                            emit_transposes(t, tiles[t], t, tile_ng[t],
                                            evac_eng="act")
                    agbuf = small.tile([128, n_cores, 64], f32, tag="agbuf")
                    ag_src = ag_out[:, :].rearrange("(s p) c -> p s c", s=n_cores, p=64)
                    nc.sync.dma_start(agbuf[0:64], ag_src)
                    nc.sync.dma_start(agbuf[64:128], ag_src)
                    red = small.tile([128, 64], f32, tag="red")
                    nc.vector.tensor_reduce(
                        red, agbuf.rearrange("p s c -> p c s"),
                        mybir.AxisListType.X, mybir.AluOpType.add,
                    )
                    w_tmp = small.tile([128, 64], f32, tag="wtmp")
                    nc.gpsimd.tensor_mul(w_tmp, red, mh2m)
                    nc.vector.tensor_sub(
                        wblk[0:64, 0:64], eye15[0:64, 0:64], w_tmp[0:64, :]
                    )
                    nc.gpsimd.tensor_sub(
                        wblk[64:128, 64:128], eye15[64:128, 64:128],
                        w_tmp[64:128, :],
                    )
                else:
                    if coll == "arsh":
                        g_out = dram.tile([64, 64], f32, tag="gosh",
                                          addr_space="Shared")
                    if collective:
                        nc.gpsimd.collective_compute(
                            "AllReduce",
                            mybir.AluOpType.add,
                            replica_groups=[list(range(n_cores))],
                            ins=[g_in.opt()],
                            outs=[g_out.opt()],
                        )
                    else:
                        nc.gpsimd.dma_start(g_out[:], g_in[:])

                    # transposes that hide under the collective
                    mid_end = min(a_tr + mid_tr, n_tiles)
                    if "tr" not in ablate:
                        for t in range(a_tr, mid_end):
                            emit_transposes(t, tiles[t], t, tile_ng[t],
                                            evac_eng="act")

                    # W0 = 1.5 I - Omega(G)/m on block-diagonal 128x128
                    nc.sync.dma_start(a_blk[0:64, 0:64], g_out[:])
                    nc.sync.dma_start(a_blk[64:128, 64:128], g_out[:])
                if coll in ("ag", "agsh"):
                    pass
                elif n_iters == 0:
                    t0_sb = small.tile([128, 128], f32, tag="w0t")
                    nc.gpsimd.tensor_mul(t0_sb, a_blk, mhalf_m)
                    nc.gpsimd.tensor_sub(wblk, eye15, t0_sb)
                else:
                    aa = small.tile([128, 128], f32, tag="aa")
                    nc.vector.tensor_scalar_mul(aa, a_blk, 1.0 / m_scale)
                    w_sb = small.tile([128, 128], f32, tag="w0")
                    nc.vector.tensor_mul(w_sb, aa, mhalf)
                    nc.vector.tensor_sub(w_sb, eye15, w_sb)
                for _it in range(n_iters):
                    t1_ps = ps_it.tile([128, 128], f32, tag="itps")
                    nc.tensor.matmul(t1_ps, aa, w_sb, start=True, stop=True)  # A W
                    t1_sb = small.tile([128, 128], f32, tag="itsb")
                    nc.vector.tensor_copy(t1_sb, t1_ps)
                    f_ps = ps_it.tile([128, 128], f32, tag="itps")
                    nc.tensor.matmul(f_ps, w_sb, t1_sb, start=True, stop=True)  # W^T A W
                    u_sb = small.tile([128, 128], f32, tag="itsb")
                    nc.vector.tensor_mul(u_sb, f_ps, mhalf)
                    nc.vector.tensor_sub(u_sb, u_sb, ehalf)  # Omega(F - I)
                    wt_ps = ps_it.tile([128, 128], f32, tag="itps")
                    nc.tensor.transpose(wt_ps, w_sb, eyef)
                    wt_sb = small.tile([128, 128], f32, tag="itsb")
                    nc.vector.tensor_copy(wt_sb, wt_ps)
                    dw_ps = ps_it.tile([128, 128], f32, tag="itps")
                    nc.tensor.matmul(dw_ps, wt_sb, u_sb, start=True, stop=True)  # W U
                    w2 = small.tile([128, 128], f32, tag="itsb2")
                    nc.vector.tensor_sub(w2, w_sb, dw_ps)
                    w_sb = w2
                if n_iters > 0:
                    nc.vector.tensor_copy(wblk, w_sb)

                # ---- Phase B: Q^T = W^T Phi^T, leftover transposes woven in ----
                # two small leading stages so the out-DMA direction ramps early
                next_b = mid_end
                bounds = [0, 256, 768]
                while bounds[-1] < xcols_out:
                    bounds.append(min(bounds[-1] + STAGE, xcols_out))
                for si, (s0, s1) in enumerate(
                    zip(bounds[:-1], bounds[1:]) if "p3" not in ablate else ()
                ):
                    if "tr" not in ablate and next_b < n_tiles:
                        emit_transposes(next_b, tiles[next_b], next_b,
                                        tile_ng[next_b])
                        next_b += 1
                    sw = s1 - s0
                    stage = outp.tile([128, STAGE], f16)
                    for b0 in range(0, sw, 512):
                        bw = min(512, sw - b0)
                        qt_ps = ps_qt.tile([128, 512], f32)
                        nc.tensor.matmul(
                            qt_ps[:, :bw],
                            wblk,
                            phit[:, s0 + b0 : s0 + b0 + bw],
                            start=True,
                            stop=True,
                        )
                        # alternate Q^T evacs between DVE and ACT
                        if (b0 // 512) % 2 == 0:
                            nc.vector.tensor_copy(stage[:, b0 : b0 + bw], qt_ps[:, :bw])
                        else:
                            nc.scalar.copy(stage[:, b0 : b0 + bw], qt_ps[:, :bw])
                    nc.sync.dma_start(qt[:, s0 : s0 + sw], stage[:, :sw])
                if "p3" in ablate and "tr" not in ablate:
                    while next_b < n_tiles:
                        emit_transposes(next_b, tiles[next_b], next_b,
                                        tile_ng[next_b])
                        next_b += 1

    nc.compile()
    return nc


def _get_nc():
    if "nc" not in _CACHE:
        _CACHE["nc"] = _build_nc()
    return _CACHE["nc"]


def _host_consts():
    identh = np.eye(128, dtype=np.float16)
    eye = np.eye(128, dtype=np.float32)
    m64 = np.triu(np.ones((64, 64), np.float32), 1) + 0.5 * np.eye(64, dtype=np.float32)
    mhalf = np.zeros((128, 128), np.float32)
    mhalf[0:64, 0:64] = m64
    mhalf[64:128, 64:128] = m64
    mh2m = np.concatenate([m64, m64], axis=0) / M_FULL
    consts = np.concatenate(
        [eye, 1.5 * eye, mhalf / M_FULL, 0.5 * eye, mhalf, mh2m], axis=1
    )
    return identh, np.ascontiguousarray(consts)


def make_in_maps(Phi: np.ndarray):
    Phi16 = np.asarray(Phi, dtype=np.float16)
    identh, consts = _host_consts()
    in_maps = []
    for c in range(N_CORES):
        shard = np.zeros((R_PAD, KAP), np.float16)
        shard[:R_CORE] = Phi16[c * R_CORE : (c + 1) * R_CORE]
        in_maps.append({"phi": shard, "identh": identh, "consts128": consts})
    return in_maps


def _decode_qt(qt_c: np.ndarray) -> np.ndarray:
    # qt[j'*64+c, ((gg*2+half)*128+p)] = sqrt(m) * Q[512*gg + 4p + 2*half + j', c]
    arr = qt_c.reshape(2, 64, N_GROUPS, 2, 128)          # [j', c, gg, half, p]
    arr = arr.transpose(2, 4, 3, 0, 1)                   # [gg, p, half, j', c]
    return arr.reshape(R_PAD, KAP)


def kernel(Phi: np.ndarray) -> np.ndarray:
    from concourse.bass_utils import run_bass_kernel_spmd

    Phi = np.asarray(Phi)
    assert Phi.shape == (M_FULL, KAP)
    nc = _get_nc()
    in_maps = make_in_maps(Phi)

    res = run_bass_kernel_spmd(nc, in_maps, core_ids=list(range(N_CORES)))
    _CACHE["last_results"] = res

    q = np.empty((M_FULL, KAP), np.float32)
    scale = np.float32(1.0 / np.sqrt(M_FULL))
    for c in range(N_CORES):
        qt_c = res.results[c]["qt"]
        dec = _decode_qt(qt_c)[:R_CORE].astype(np.float32)
        q[c * R_CORE : (c + 1) * R_CORE] = dec * scale
    return q


# revision 30
# speedup vs baseline: 1.4318x; 1.4318x over previous
"""Trainium2 Bass kernel for nn_Basis: Gram-Schmidt orthonormalization of a
500000x64 matrix across 8 NeuronCores.

Math: classical Gram-Schmidt of a (very well conditioned) Gaussian matrix
equals the QR Q-factor with positive-diagonal R, which equals Phi @ W where
W = R^{-1} = inverse Cholesky factor of G = Phi^T Phi.  W is computed with a
quadratically-convergent coupled iteration on the Gram matrix (no 64-step
sequential scan):  A = G/m = I + E with ||E|| ~ 2%, iterate
W <- W (I - Omega(W^T A W - I)) where Omega(F) = strict_upper(F) + diag(F)/2.
One iteration reaches ~1e-6; the fp16 data path dominates the error budget.

v2 (fp16 I/O): Phi is cast to fp16 on the host, halving DMA-in; Q^T leaves
the chip as fp16 (unscaled by sqrt(m) so values are O(1)), halving DMA-out.
Rows are packed 4 per partition (GROUP=512) so fp16 DMA chunks stay 512B.
The Gram accumulates into a single 64x64 PSUM tile via 4 narrow matmuls per
group (no even/odd block merge).  The Newton iteration runs on 128x128
block-diagonal operands so the stationary matrix for phase 3 comes out in
block form directly.  PE transposes of Phi tiles are split between phase A
(load), the AllReduce window, and the phase-3 stage loop so every phase
stays under the DMA roofline.  The host de-interleaves and scales output.
"""
import sys

sys.path.insert(0, "/opt/trn_rl_repo")

import numpy as np

N_CORES = 8
M_FULL = 500000
KAP = 64
R_CORE = M_FULL // N_CORES          # 62500 rows per core
GROUP = 512                          # rows per packed group (4 per partition)
TILE_G = 4                           # groups per DMA tile
N_TILES = 31
N_GROUPS = N_TILES * TILE_G          # 124
R_PAD = N_GROUPS * GROUP             # 63488 padded rows per core
XCOLS = N_GROUPS * 2 * 128           # 31744 columns in phit / qt
STAGE = 2048                         # output columns per staged DMA
A_TR = 12                            # tiles transposed inline during load
MID_TR = 10                          # tiles transposed during the AllGather
N_ITERS = 0

_CACHE = {}


COLL_MODE = "agsh"


def _build_nc(n_tiles=N_TILES, m_scale=float(M_FULL), n_cores=N_CORES,
              collective=True, ablate=(), repeat=1,
              a_tr=A_TR, mid_tr=MID_TR, n_iters=N_ITERS, coll=None):
    if coll is None:
        coll = COLL_MODE
    if n_iters > 0 or not collective:
        coll = "ar"
    import concourse.mybir as mybir
    from concourse import bacc, tile

    r_pad = n_tiles * TILE_G * GROUP
    xcols = n_tiles * TILE_G * 2 * 128
    # The last group (rows 62976..63487) is pure zero padding: skip its load,
    # gram, transposes and output columns. Host decode never reads past row
    # 62975 (group 122), and the donated output buffer zero-fills the tail.
    skip_last = n_tiles == N_TILES and "p3" not in ablate
    xcols_out = xcols - (256 if skip_last else 0)

    f32 = mybir.dt.float32
    f16 = mybir.dt.float16
    bf16 = mybir.dt.bfloat16
    nc = bacc.Bacc(trn_type="TRN2", target_bir_lowering=False, debug=False)

    phi = nc.declare_dram_parameter("phi", [r_pad, KAP], f16, isOutput=False)
    identh = nc.declare_dram_parameter("identh", [128, 128], f16, isOutput=False)
    # consts128 cols: [0:128]=I(f32), [128:256]=1.5I, [256:384]=blockdiag
    # Omega mask (strict upper 1.0, diag 0.5) scaled by 1/m, [384:512]=0.5I,
    # [512:640]=unscaled Omega mask (Newton iterations only),
    # [640:704]=vertically stacked [Omega/m; Omega/m] for the allgather path
    consts = nc.declare_dram_parameter("consts128", [128, 704], f32, isOutput=False)
    qt = nc.declare_dram_parameter("qt", [128, xcols], f16, isOutput=True)

    from concourse.bass import ds

    # phi rows: row = 512*(TILE_G*t+g) + 4p + j, col c -> per tile [p, g, (j c)]
    def phi_tile_ap(t, ng=TILE_G):
        return phi[ds(t * TILE_G * GROUP, ng * GROUP), :].rearrange(
            "(g p j) c -> p g (j c)", g=ng, p=128, j=4
        )

    with tile.TileContext(nc) as tc:
        with (
            tc.tile_pool(name="consts", bufs=1) as cpool,
            tc.tile_pool(name="persist", bufs=1) as persist,
            tc.tile_pool(name="inp", bufs=n_tiles) as inp,
            tc.tile_pool(name="outp", bufs=3) as outp,
            tc.tile_pool(name="small", bufs=2) as small,
            tc.tile_pool(name="ps_gram", bufs=1, space="PSUM") as ps_gram,
            tc.tile_pool(name="ps_tr", bufs=2, space="PSUM") as ps_tr,
            tc.tile_pool(name="ps_it", bufs=2, space="PSUM") as ps_it,
            tc.tile_pool(name="ps_qt", bufs=3, space="PSUM") as ps_qt,
            tc.tile_pool(name="dram", bufs=1, space="DRAM") as dram,
        ):
            ident_sb = cpool.tile([128, 128], f16)
            nc.sync.dma_start(ident_sb, identh[:, :])
            consts_sb = cpool.tile([128, 704], f32)
            nc.sync.dma_start(consts_sb, consts[:, :])
            eyef = consts_sb[:, 0:128]
            eye15 = consts_sb[:, 128:256]
            mhalf_m = consts_sb[:, 256:384]
            ehalf = consts_sb[:, 384:512]
            mhalf = consts_sb[:, 512:640]
            mh2m = consts_sb[:, 640:704]

            for _rep in range(repeat):
                phit = persist.tile([128, xcols], f16)
                a_blk = persist.tile([128, 128], f32, tag="ablk")
                nc.any.memzero(a_blk)
                g64 = ps_gram.tile([64, 64], f32)

                # ---- Phase A: load + gram; transpose first a_tr tiles ----
                n_groups = n_tiles * TILE_G
                gi = 0

                def emit_transposes(t, in_t, evac_parity, ng=TILE_G,
                                    evac_eng=None):
                    # 2 transposes per group -> 2 PSUM tiles -> 2 evacs.
                    # evac_eng None alternates DVE/ACT; "act" keeps DVE clear
                    # (used in the collective window so the reduce chain isn't
                    # queued behind evacuations on DVE).
                    for half_t in range(2):
                        nk = min(4, 2 * ng - 4 * half_t)
                        if nk <= 0:
                            break
                        tr_ps = ps_tr.tile([128, 512], f16)
                        for k in range(nk):
                            g = half_t * 2 + k // 2
                            h = k % 2
                            nc.tensor.transpose(
                                tr_ps[:, 128 * k : 128 * (k + 1)],
                                in_t[:, g, 128 * h : 128 * (h + 1)],
                                ident_sb,
                            )
                        x0 = 1024 * t + 512 * half_t
                        dst = phit[:, x0 : x0 + 128 * nk]
                        use_dve = (
                            evac_eng is None and (evac_parity + half_t) % 2 == 0
                        )
                        if use_dve:
                            nc.vector.tensor_copy(dst, tr_ps[:, : 128 * nk])
                        else:
                            nc.scalar.copy(dst, tr_ps[:, : 128 * nk])

                tiles = []
                tile_ng = [
                    TILE_G - 1 if (skip_last and t == n_tiles - 1) else TILE_G
                    for t in range(n_tiles)
                ]
                n_gram = 4 * sum(tile_ng)
                for t in range(n_tiles):
                    ng = tile_ng[t]
                    in_t = inp.tile([128, TILE_G, 256], f16)
                    nc.sync.dma_start(in_t[:, :ng, :], phi_tile_ap(t, ng))
                    tiles.append(in_t)
                    for g in range(ng):
                        for j in range(4):
                            if "gram" in ablate:
                                gi += 1
                                continue
                            nc.tensor.matmul(
                                g64,
                                in_t[:, g, 64 * j : 64 * j + 64],
                                in_t[:, g, 64 * j : 64 * j + 64],
                                start=(gi == 0),
                                stop=(gi == n_gram - 1),
                            )
                            gi += 1
                    if "tr" not in ablate and t < a_tr:
                        emit_transposes(t, tiles[t], t, ng)

                # ---- combine the per-core 64x64 Grams across cores ----
                cdt = bf16 if coll in ("ag", "agsh") else f32
                g_in = dram.tile([64, 64], cdt)
                g_out = dram.tile([64, 64], f32)
                wblk = persist.tile([128, 128], f16, tag="wblk")
                if coll in ("ag", "agsh"):
                    nc.any.memzero(wblk)
                if "gram" not in ablate:
                    g_sb = small.tile([64, 64], cdt, tag="gsb")
                    nc.vector.tensor_copy(g_sb, g64)
                    nc.sync.dma_start(g_in[:], g_sb)
                if coll in ("ag", "agsh"):
                    # AllGather + on-chip reduce: one collective phase instead
                    # of reduce-scatter+gather, and no DRAM round trip for the
                    # block-diagonal A.
                    ag_out = dram.tile(
                        [64 * n_cores, 64], bf16, tag="agout",
                        addr_space="Shared" if coll == "agsh" else "Local",
                    )
                    nc.gpsimd.collective_compute(
                        "AllGather",
                        mybir.AluOpType.bypass,
                        replica_groups=[list(range(n_cores))],
                        ins=[g_in.opt()],
                        outs=[ag_out.opt()],
                    )
                    mid_end = min(a_tr + mid_tr, n_tiles)
                    if "tr" not in ablate:
                        for t in range(a_tr, mid_end):
                            emit_transposes(t, tiles[t], t, tile_ng[t],
                                            evac_eng="act")
                    agbuf = small.tile([128, n_cores, 64], bf16, tag="agbuf")
                    ag_src = ag_out[:, :].rearrange("(s p) c -> p s c", s=n_cores, p=64)
                    nc.sync.dma_start(agbuf[0:64], ag_src)
                    nc.sync.dma_start(agbuf[64:128], ag_src)
                    red = small.tile([128, 64], f32, tag="red")
                    nc.vector.tensor_reduce(
                        red, agbuf.rearrange("p s c -> p c s"),
                        mybir.AxisListType.X, mybir.AluOpType.add,
                    )
                    w_tmp = small.tile([128, 64], f32, tag="wtmp")
                    nc.gpsimd.tensor_mul(w_tmp, red, mh2m)
                    nc.vector.tensor_sub(
                        wblk[0:64, 0:64], eye15[0:64, 0:64], w_tmp[0:64, :]
                    )
                    nc.gpsimd.tensor_sub(
                        wblk[64:128, 64:128], eye15[64:128, 64:128],
                        w_tmp[64:128, :],
                    )
                else:
                    if coll == "arsh":
                        g_out = dram.tile([64, 64], f32, tag="gosh",
                                          addr_space="Shared")
                    if collective:
                        nc.gpsimd.collective_compute(
                            "AllReduce",
                            mybir.AluOpType.add,
                            replica_groups=[list(range(n_cores))],
                            ins=[g_in.opt()],
                            outs=[g_out.opt()],
                        )
                    else:
                        nc.gpsimd.dma_start(g_out[:], g_in[:])

                    # transposes that hide under the collective
                    mid_end = min(a_tr + mid_tr, n_tiles)
                    if "tr" not in ablate:
                        for t in range(a_tr, mid_end):
                            emit_transposes(t, tiles[t], t, tile_ng[t],
                                            evac_eng="act")

                    # W0 = 1.5 I - Omega(G)/m on block-diagonal 128x128
                    nc.sync.dma_start(a_blk[0:64, 0:64], g_out[:])
                    nc.sync.dma_start(a_blk[64:128, 64:128], g_out[:])
                if coll in ("ag", "agsh"):
                    pass
                elif n_iters == 0:
                    t0_sb = small.tile([128, 128], f32, tag="w0t")
                    nc.gpsimd.tensor_mul(t0_sb, a_blk, mhalf_m)
                    nc.gpsimd.tensor_sub(wblk, eye15, t0_sb)
                else:
                    aa = small.tile([128, 128], f32, tag="aa")
                    nc.vector.tensor_scalar_mul(aa, a_blk, 1.0 / m_scale)
                    w_sb = small.tile([128, 128], f32, tag="w0")
                    nc.vector.tensor_mul(w_sb, aa, mhalf)
                    nc.vector.tensor_sub(w_sb, eye15, w_sb)
                for _it in range(n_iters):
                    t1_ps = ps_it.tile([128, 128], f32, tag="itps")
                    nc.tensor.matmul(t1_ps, aa, w_sb, start=True, stop=True)  # A W
                    t1_sb = small.tile([128, 128], f32, tag="itsb")
                    nc.vector.tensor_copy(t1_sb, t1_ps)
                    f_ps = ps_it.tile([128, 128], f32, tag="itps")
                    nc.tensor.matmul(f_ps, w_sb, t1_sb, start=True, stop=True)  # W^T A W
                    u_sb = small.tile([128, 128], f32, tag="itsb")
                    nc.vector.tensor_mul(u_sb, f_ps, mhalf)
                    nc.vector.tensor_sub(u_sb, u_sb, ehalf)  # Omega(F - I)
                    wt_ps = ps_it.tile([128, 128], f32, tag="itps")
                    nc.tensor.transpose(wt_ps, w_sb, eyef)
                    wt_sb = small.tile([128, 128], f32, tag="itsb")
                    nc.vector.tensor_copy(wt_sb, wt_ps)
                    dw_ps = ps_it.tile([128, 128], f32, tag="itps")
                    nc.tensor.matmul(dw_ps, wt_sb, u_sb, start=True, stop=True)  # W U
                    w2 = small.tile([128, 128], f32, tag="itsb2")
                    nc.vector.tensor_sub(w2, w_sb, dw_ps)
                    w_sb = w2
                if n_iters > 0:
                    nc.vector.tensor_copy(wblk, w_sb)

                # ---- Phase B: Q^T = W^T Phi^T, leftover transposes woven in ----
                # two small leading stages so the out-DMA direction ramps early
                next_b = mid_end
                bounds = [0, 256, 768]
                while bounds[-1] < xcols_out:
                    bounds.append(min(bounds[-1] + STAGE, xcols_out))
                for si, (s0, s1) in enumerate(
                    zip(bounds[:-1], bounds[1:]) if "p3" not in ablate else ()
                ):
                    if "tr" not in ablate and next_b < n_tiles:
                        emit_transposes(next_b, tiles[next_b], next_b,
                                        tile_ng[next_b])
                        next_b += 1
                    sw = s1 - s0
                    stage = outp.tile([128, STAGE], f16)
                    for b0 in range(0, sw, 512):
                        bw = min(512, sw - b0)
                        qt_ps = ps_qt.tile([128, 512], f32)
                        nc.tensor.matmul(
                            qt_ps[:, :bw],
                            wblk,
                            phit[:, s0 + b0 : s0 + b0 + bw],
                            start=True,
                            stop=True,
                        )
                        # alternate Q^T evacs between DVE and ACT
                        if (b0 // 512) % 2 == 0:
                            nc.vector.tensor_copy(stage[:, b0 : b0 + bw], qt_ps[:, :bw])
                        else:
                            nc.scalar.copy(stage[:, b0 : b0 + bw], qt_ps[:, :bw])
                    nc.sync.dma_start(qt[:, s0 : s0 + sw], stage[:, :sw])
                if "p3" in ablate and "tr" not in ablate:
                    while next_b < n_tiles:
                        emit_transposes(next_b, tiles[next_b], next_b,
                                        tile_ng[next_b])
                        next_b += 1

    nc.compile()
    return nc


def _get_nc():
    if "nc" not in _CACHE:
        _CACHE["nc"] = _build_nc()
    return _CACHE["nc"]


def _host_consts():
    identh = np.eye(128, dtype=np.float16)
    eye = np.eye(128, dtype=np.float32)
    m64 = np.triu(np.ones((64, 64), np.float32), 1) + 0.5 * np.eye(64, dtype=np.float32)
    mhalf = np.zeros((128, 128), np.float32)
    mhalf[0:64, 0:64] = m64
    mhalf[64:128, 64:128] = m64
    mh2m = np.concatenate([m64, m64], axis=0) / M_FULL
    consts = np.concatenate(
        [eye, 1.5 * eye, mhalf / M_FULL, 0.5 * eye, mhalf, mh2m], axis=1
    )
    return identh, np.ascontiguousarray(consts)


def make_in_maps(Phi: np.ndarray):
    Phi16 = np.asarray(Phi, dtype=np.float16)
    identh, consts = _host_consts()
    in_maps = []
    for c in range(N_CORES):
        shard = np.zeros((R_PAD, KAP), np.float16)
        shard[:R_CORE] = Phi16[c * R_CORE : (c + 1) * R_CORE]
        in_maps.append({"phi": shard, "identh": identh, "consts128": consts})
    return in_maps


def _decode_qt(qt_c: np.ndarray) -> np.ndarray:
    # qt[j'*64+c, ((gg*2+half)*128+p)] = sqrt(m) * Q[512*gg + 4p + 2*half + j', c]
    arr = qt_c.reshape(2, 64, N_GROUPS, 2, 128)          # [j', c, gg, half, p]
    arr = arr.transpose(2, 4, 3, 0, 1)                   # [gg, p, half, j', c]
    return arr.reshape(R_PAD, KAP)


def kernel(Phi: np.ndarray) -> np.ndarray:
    from concourse.bass_utils import run_bass_kernel_spmd

    Phi = np.asarray(Phi)
    assert Phi.shape == (M_FULL, KAP)
    nc = _get_nc()
    in_maps = make_in_maps(Phi)

    res = run_bass_kernel_spmd(nc, in_maps, core_ids=list(range(N_CORES)))
    _CACHE["last_results"] = res

    q = np.empty((M_FULL, KAP), np.float32)
    scale = np.float32(1.0 / np.sqrt(M_FULL))
    for c in range(N_CORES):
        qt_c = res.results[c]["qt"]
        dec = _decode_qt(qt_c)[:R_CORE].astype(np.float32)
        q[c * R_CORE : (c + 1) * R_CORE] = dec * scale
    return q
